# revision 34
# baseline (speedup 1.0000x reference)
"""Trainium2 Bass kernel for nn_PolicyNetwork3 (2-layer GraphSAGE + edge-MLP).

Design (8 NeuronCores, SPMD single NEFF):
- dst-sharded aggregation: core k owns node block [6272k, 6272k+6272).
- Edges sorted by (core, dst-window of 128, src-half); messages gathered from
  bf16 HBM row tables via dma_gather (int16 idx; uneven LO/HI table split at
  row 28672 so both halves fit int16 and align to AllGather chunks).
- segment-sum per 128-dst window via matmuls against host-precomputed bf16
  invdeg-one-hot blocks streamed from HBM; PSUM accumulation per window.
- BN folded into the SAGE weights on host; leaky-relu on the Scalar engine.
- Global node tables (xpad/hfull/gqfull) use a chunk-interleaved row layout:
  row(n) = (l//896)*7168 + core*896 + (l%896), so the h/gq shard exchange runs
  as 7 chunked AllGathers that overlap the gather stream; only the last chunk
  sits on the critical path.
- candidate MLP folds its first layer into per-node tables [g|q|q|g]; u/v rows
  are fetched with transposed gathers ([feat, slot] layout) and the MLP runs
  as wide PE matmuls over 512-slot tiles; global softmax after an AllGather.
"""

import sys

sys.path.insert(0, "/opt/trn_rl_repo")
sys.path.insert(0, "/root/.axon_site")

import ml_dtypes
import numpy as np

import concourse.bacc as bacc
import concourse.bass as bass
import concourse.bass_isa as bass_isa
import concourse.mybir as mybir
import concourse.tile as tile
from concourse import library_config
from concourse.bass_utils import run_bass_kernel_spmd

P = 128
N, E, C = 50000, 800000, 100000
D = 128
NCORE = 8
NSH = 6272          # nodes per core shard
NTOT = NSH * NCORE  # 50176 padded node table
WIN = 128           # dst nodes per aggregation window
NWIN = NSH // WIN   # 49 windows == linear blocks per core
NBLK = NWIN
NCHK = 7            # AllGather chunks per shard
CJ = NSH // NCHK    # 896 rows per core per chunk
GJ = NCORE * CJ     # 7168 global rows per chunk
LOB = 4 * GJ        # 28672: LO/HI gather-table split (both halves < 2^15)
CSH = C // NCORE    # 12500 candidates per core
GCALL = 4096        # max idxs per dma_gather call
AGSLACK = 3         # extra gather calls before an AG trigger
BN_EPS = 1e-5
SLOPE = 0.01
F32 = mybir.dt.float32
BF16 = mybir.dt.bfloat16
I16 = mybir.dt.int16
AF = mybir.ActivationFunctionType
ALU = mybir.AluOpType
NPBF = ml_dtypes.bfloat16


def _newrow(n):
    """Original padded node id -> chunk-interleaved global table row.

    Window-group g (windows 7g..7g+6, completing g-th in stream order) feeds
    AG chunk (g+4)%7, so the HI half of the table (chunks 4-6) is ready
    earliest -- the next layer's gather stream starts with HI calls."""
    k = n // NSH
    l = n - k * NSH
    return ((l // CJ + 4) % NCHK) * GJ + k * CJ + (l % CJ)


def _wrap16(idx_lin):
    """[n] -> [128, n/16] int16 in the dma_gather wrapped+replicated layout."""
    n = idx_lin.shape[0]
    assert n % 16 == 0
    w = idx_lin.reshape(n // 16, 16).T.astype(np.int16)
    return np.tile(w, (8, 1)).copy()


def gidx_to_cols(arr):
    """[nslot] -> [128, nchunk] with slot i at [i%128, i//128]."""
    n = arr.shape[0]
    return arr.reshape(n // P, P).T.copy()


def _prep_edges(src, dst, invdeg):
    """Build the uniform per-core chunk schedule + per-core index data."""
    core = dst // NSH
    dl = dst - core * NSH
    winl = dl // WIN
    srow = _newrow(src)
    half = (srow >= LOB).astype(np.int64)
    key = (core * NWIN + winl) * 2 + half
    order = np.argsort(key, kind="stable")
    cnt = np.bincount(key, minlength=NCORE * NWIN * 2).reshape(NCORE, NWIN * 2)
    nch = -(-cnt // P)                       # ceil chunks per (core, win*2+half)
    nch_u = nch.max(axis=0)                  # [NWIN*2] uniform chunk counts
    # stream order: all HI runs (win 0..48), then all LO runs; windows then
    # complete spread across the longer LO pass, spacing the AG triggers
    runs = [(w, s) for s in (1, 0) for w in range(NWIN)]
    run_nch = [int(nch_u[w * 2 + s]) for (w, s) in runs]
    tot_ch = sum(run_nch)
    nslot = tot_ch * P

    gidx = np.zeros((NCORE, nslot), np.int16)
    dstloc = np.full((NCORE, nslot), -5, np.int64)
    val = np.zeros((NCORE, nslot), np.float32)
    bstart = np.zeros(NCORE * NWIN * 2 + 1, np.int64)
    np.cumsum(np.bincount(key, minlength=NCORE * NWIN * 2), out=bstart[1:])
    for k in range(NCORE):
        pos = 0
        for (w, s) in runs:
            b = (k * NWIN + w) * 2 + s
            e0, e1 = bstart[b], bstart[b + 1]
            n = e1 - e0
            sl = order[e0:e1]
            sl = sl[np.argsort(srow[sl], kind="stable")]
            gidx[k, pos : pos + n] = (srow[sl] - s * LOB).astype(np.int16)
            dstloc[k, pos : pos + n] = dl[sl] % WIN
            val[k, pos : pos + n] = invdeg[dst[sl]]
            pos += nch_u[w * 2 + s] * P
    # gather call split: contiguous HI slots then LO slots, calls <= GCALL
    n_hi = sum(run_nch[:NWIN]) * P
    calls = []  # (slot_start, n_idx, half)
    for s, lo, hi in ((1, 0, n_hi), (0, n_hi, nslot)):
        p = lo
        while p < hi:
            n = min(GCALL, hi - p)
            calls.append((p, n, s))
            p += n
    # per-run chunk offsets
    run_off = np.zeros(len(runs) + 1, np.int64)
    np.cumsum(run_nch, out=run_off[1:])
    # AG trigger schedule: AG chunk q needs windows 7q..7q+6 complete (their
    # HI runs); trigger after the gather call covering that slot, plus slack.
    agcall = []
    for q in range(NCHK):
        end_slot = run_off[NWIN + 7 * q + 7] * P  # end of HI run of win 7q+6
        ci = next(i for i, (s0, n, s) in enumerate(calls) if s0 + n >= end_slot)
        agcall.append(min(ci + AGSLACK, len(calls) - 1))
    meta = dict(runs=runs, run_nch=run_nch, run_off=run_off, tot_ch=tot_ch,
                nslot=nslot, calls=calls, agcall=agcall)
    data = []
    for k in range(NCORE):
        # one-hot scatter table: slot i = chunk i//128, partition i%128
        oh = np.zeros((tot_ch, P, WIN), np.float32)
        sl = np.nonzero(dstloc[k] >= 0)[0]
        oh[sl // P, sl % P, dstloc[k][sl]] = val[k][sl]
        data.append(dict(gidx=_wrap16(gidx[k]),
                         ohtab=oh.transpose(1, 0, 2).reshape(P, tot_ch * WIN)
                         .astype(NPBF)))
    return meta, data


def _prep_cands(cand_u, cand_v, cand_feat):
    """Shard candidates, group by (u_half, v_half), pad to uniform chunks."""
    urow = _newrow(cand_u)
    vrow = _newrow(cand_v)
    percore = [np.arange(k * CSH, (k + 1) * CSH) for k in range(NCORE)]
    groups = [[None] * 4 for _ in range(NCORE)]
    for k in range(NCORE):
        ids = percore[k]
        g = (urow[ids] >= LOB) * 2 + (vrow[ids] >= LOB)
        o = np.argsort(g, kind="stable")
        ids = ids[o]
        gs = g[o]
        for gi in range(4):
            gids = ids[gs == gi]
            groups[k][gi] = gids[np.argsort(urow[gids], kind="stable")]
    gch = np.zeros((NCORE, 4), np.int64)
    for k in range(NCORE):
        for gi in range(4):
            gch[k, gi] = -(-len(groups[k][gi]) // P)
    gorder = [3, 2, 1, 0]
    gch_u = gch.max(axis=0)                 # uniform chunks per group
    ncc = int(gch_u.sum())
    cslot = ncc * P
    cu = np.zeros((NCORE, cslot), np.int16)
    cv = np.zeros((NCORE, cslot), np.int16)
    ft = np.zeros((NCORE, cslot), np.float32)
    mask = np.full((NCORE, cslot), -1e30, np.float32)
    slotmap = np.full((NCORE, cslot), -1, np.int64)
    goff = np.zeros(5, np.int64)
    np.cumsum(gch_u[gorder] * P, out=goff[1:])
    gpos = {gi: goff[i] for i, gi in enumerate(gorder)}
    for k in range(NCORE):
        for gi in range(4):
            ids = groups[k][gi]
            n = len(ids)
            p0 = gpos[gi]
            uh, vh = gi // 2, gi % 2
            cu[k, p0 : p0 + n] = (urow[ids] - uh * LOB).astype(np.int16)
            cv[k, p0 : p0 + n] = (vrow[ids] - vh * LOB).astype(np.int16)
            ft[k, p0 : p0 + n] = cand_feat[ids, 0]
            mask[k, p0 : p0 + n] = 0.0
            slotmap[k, p0 : p0 + n] = ids
    # gather calls: u -> runs (groups 3-2 HI | 1-0 LO); v -> one run per group
    ucalls, vcalls = [], []
    for s, lo, hi in ((1, goff[0], goff[2]), (0, goff[2], goff[4])):
        p = lo
        while p < hi:
            n = min(GCALL, hi - p)
            ucalls.append((int(p), int(n), s))
            p += n
    for i, gi in enumerate(gorder):
        p, hi = goff[i], goff[i + 1]
        while p < hi:
            n = min(GCALL, hi - p)
            vcalls.append((int(p), int(n), gi % 2))
            p += n
    meta = dict(ncc=ncc, cslot=cslot, ucalls=ucalls, vcalls=vcalls)
    data = [dict(cu=_wrap16(cu[k]), cv=_wrap16(cv[k]),
                 featrow=ft[k][None, :].astype(NPBF),
                 mask=gidx_to_cols(mask[k]),
                 slotmap=slotmap[k]) for k in range(NCORE)]
    return meta, data


def _build_nc(em, cm):
    nc = bacc.Bacc("TRN2", target_bir_lowering=False, debug=False,
                   num_devices=NCORE)
    f32 = F32
    TOTCH, NSLOT = em["tot_ch"], em["nslot"]
    NCC, CSLOT = cm["ncc"], cm["cslot"]

    # ---- external inputs ----
    xpad = nc.dram_tensor("xpad", [NTOT, D], BF16, kind="ExternalInput")
    xT = nc.dram_tensor("xT", [P, NSH], f32, kind="ExternalInput")
    gidx = nc.dram_tensor("gidx", [P, NSLOT // 16], I16, kind="ExternalInput")
    ohtab = nc.dram_tensor("ohtab", [P, TOTCH * WIN], BF16, kind="ExternalInput")
    wself = [nc.dram_tensor(f"wself{l}", [D, D], f32, kind="ExternalInput") for l in range(2)]
    wneigh = [nc.dram_tensor(f"wneigh{l}", [D, D], f32, kind="ExternalInput") for l in range(2)]
    crow = [nc.dram_tensor(f"crow{l}", [1, D], f32, kind="ExternalInput") for l in range(2)]
    ident = nc.dram_tensor("ident", [P, P], f32, kind="ExternalInput")
    onesr = nc.dram_tensor("onesr", [1, P], f32, kind="ExternalInput")
    amat = nc.dram_tensor("amat", [D, 64], f32, kind="ExternalInput")
    bmat = nc.dram_tensor("bmat", [D, 64], f32, kind="ExternalInput")
    mb0r = nc.dram_tensor("mb0r", [1, 64], f32, kind="ExternalInput")
    w0r1 = nc.dram_tensor("w0r1", [1, 64], BF16, kind="ExternalInput")
    i64 = nc.dram_tensor("i64", [64, 64], BF16, kind="ExternalInput")
    mw1b = nc.dram_tensor("mw1b", [64, 64], BF16, kind="ExternalInput")
    mb1r1 = nc.dram_tensor("mb1r1", [1, 64], BF16, kind="ExternalInput")
    mw2c = nc.dram_tensor("mw2c", [64, 1], BF16, kind="ExternalInput")
    mb2v = nc.dram_tensor("mb2v", [P, 1], f32, kind="ExternalInput")
    ones512 = nc.dram_tensor("ones512", [1, 512], BF16, kind="ExternalInput")
    cu = nc.dram_tensor("cu", [P, CSLOT // 16], I16, kind="ExternalInput")
    cv = nc.dram_tensor("cv", [P, CSLOT // 16], I16, kind="ExternalInput")
    featrow = nc.dram_tensor("featrow", [1, CSLOT], BF16, kind="ExternalInput")
    maskr = nc.dram_tensor("maskr", [P, NCC], f32, kind="ExternalInput")
    # ---- outputs ----
    y_out = nc.dram_tensor("y_out", [P, NCC], f32, kind="ExternalOutput")
    p_out = nc.dram_tensor("p_out", [P, NCORE * CSLOT // P], f32, kind="ExternalOutput")
    # ---- internal DRAM ----
    hshq = [nc.dram_tensor(f"hshq{q}", [CJ, D], BF16, kind="Internal")
            for q in range(NCHK)]
    hfull = nc.dram_tensor("hfull", [NTOT, D], BF16, kind="Internal", addr_space="Shared")
    gqshq = [nc.dram_tensor(f"gqshq{q}", [CJ, 2 * D], BF16, kind="Internal")
             for q in range(NCHK)]
    gqfull = nc.dram_tensor("gqfull", [NTOT, 2 * D], BF16, kind="Internal", addr_space="Shared")
    ysh = nc.dram_tensor("ysh", [P, NCC], f32, kind="Internal")
    yfull = nc.dram_tensor("yfull", [NCORE * P, NCC], f32, kind="Internal", addr_space="Shared")

    rg = [list(range(NCORE))]

    with tile.TileContext(nc) as tc:
        with (
            tc.tile_pool(name="const", bufs=1) as cp,
            tc.tile_pool(name="big", bufs=1) as bp,
            tc.tile_pool(name="msgs", bufs=3) as mp,
            tc.tile_pool(name="oh", bufs=3) as ohp,
            tc.tile_pool(name="wrk", bufs=4) as wp,
            tc.tile_pool(name="ps_run", bufs=2, space="PSUM") as ps_run,
            tc.tile_pool(name="ps_t", bufs=2, space="PSUM") as ps_t,
            tc.tile_pool(name="ps_h", bufs=2, space="PSUM") as ps_h,
            tc.tile_pool(name="ps_s", bufs=1, space="PSUM") as ps_s,
        ):
            nc.gpsimd.load_library(library_config.mlp)

            def load(pool, t, shape=None):
                tl = pool.tile(shape or list(t.shape), t.dtype, tag=t.name)
                nc.sync.dma_start(tl[:], t[:])
                return tl

            gidx_t = load(cp, gidx)
            ident_t = load(cp, ident)
            onesr_t = load(cp, onesr)
            wself_t = [load(cp, w) for w in wself]
            wneigh_t = [load(cp, w) for w in wneigh]
            crow_t = [load(cp, w) for w in crow]
            amat_t = load(cp, amat)
            bmat_t = load(cp, bmat)
            mb0r_t = load(cp, mb0r)
            w0r1_t = load(cp, w0r1)
            i64_t = load(cp, i64)
            mw1b_t = load(cp, mw1b)
            mb1r1_t = load(cp, mb1r1)
            mw2c_t = load(cp, mw2c)
            mb2_t = load(cp, mb2v)
            ones512_t = load(cp, ones512)
            cu_t = load(cp, cu)
            cv_t = load(cp, cv)
            featrow_t = load(cp, featrow)
            mask_t = load(cp, maskr)

            curT = bp.tile([P, NSH], f32, tag="curT")
            nxtT = bp.tile([P, NSH], f32, tag="nxtT")
            aggr = bp.tile([P, NSH], f32, tag="aggr")
            nc.sync.dma_start(curT[:], xT[:])

            runs, run_nch, run_off = em["runs"], em["run_nch"], em["run_off"]
            calls, agcall = em["calls"], em["agcall"]
            # chunk -> (run_index, pos_in_run)
            ch_run = []
            for ri, nchk in enumerate(run_nch):
                ch_run += [(ri, j, nchk) for j in range(nchk)]

            def win_tail(layer, cT, nT, w):
                """Linear block + h/gq table writes once window w is final."""
                ph = ps_h.tile([P, P], f32, tag="ph")
                nc.tensor.matmul(ph[:], lhsT=aggr[:, w * P : (w + 1) * P],
                                 rhs=wneigh_t[layer][:],
                                 start=True, stop=False)
                nc.tensor.matmul(ph[:], lhsT=cT[:, w * P : (w + 1) * P],
                                 rhs=wself_t[layer][:], start=False, stop=False)
                nc.tensor.matmul(ph[:], lhsT=onesr_t[:], rhs=crow_t[layer][:],
                                 start=False, stop=True)
                ht = wp.tile([P, P], f32, tag="ht")
                nc.scalar.activation(ht[:], ph[:], AF.Lrelu, alpha=SLOPE)
                q, r = (w // NCHK + 4) % NCHK, (w % NCHK) * P
                if layer == 0:
                    htb = wp.tile([P, P], BF16, tag="htb")
                    nc.scalar.activation(htb[:], ph[:], AF.Lrelu, alpha=SLOPE)
                    nc.sync.dma_start(hshq[q][r : r + P, :], htb[:])
                pt2 = ps_t.tile([P, P], f32, tag="pt")
                nc.tensor.transpose(pt2[:], ht[:], ident_t[:])
                nc.scalar.activation(nT[:, w * P : (w + 1) * P], pt2[:], AF.Copy)
                if layer == 1:
                    # g/q tables ([g|q] cols 0:128, [q|g] cols 128:256)
                    h2T = nT[:, w * P : (w + 1) * P]
                    pg = ps_h.tile([P, 64], f32, tag="ph")
                    nc.tensor.matmul(pg[:], lhsT=h2T, rhs=amat_t[:],
                                     start=True, stop=True)
                    pq = ps_t.tile([P, 64], f32, tag="pt")
                    nc.tensor.matmul(pq[:], lhsT=h2T, rhs=bmat_t[:],
                                     start=True, stop=False)
                    nc.tensor.matmul(pq[:], lhsT=onesr_t[:], rhs=mb0r_t[:],
                                     start=False, stop=True)
                    gq = wp.tile([P, 2 * P], BF16, tag="gq")
                    nc.scalar.activation(gq[:, 0:64], pg[:], AF.Copy)
                    nc.scalar.activation(gq[:, 64:128], pq[:], AF.Copy)
                    nc.scalar.activation(gq[:, 128:192], pq[:], AF.Copy)
                    nc.scalar.activation(gq[:, 192:256], pg[:], AF.Copy)
                    nc.sync.dma_start(gqshq[q][r : r + P, :], gq[:])

            def emit_ag(layer, q):
                if layer == 0:
                    nc.gpsimd.collective_compute(
                        "AllGather", ALU.bypass, replica_groups=rg,
                        ins=[hshq[q][:].opt()],
                        outs=[hfull[q * GJ : (q + 1) * GJ, :].opt()])
                else:
                    nc.gpsimd.collective_compute(
                        "AllGather", ALU.bypass, replica_groups=rg,
                        ins=[gqshq[q][:].opt()],
                        outs=[gqfull[q * GJ : (q + 1) * GJ, :].opt()])

            for layer in range(2):
                gtab = xpad if layer == 0 else hfull
                ps = None
                for ci, (s0, n_idx, s) in enumerate(calls):
                    c0, ncall = s0 // P, n_idx // P
                    g = mp.tile([P, GCALL // P, P], BF16, tag="g")
                    nc.gpsimd.dma_gather(
                        g[:, :ncall, :],
                        gtab[s * LOB : NTOT if s else LOB, :],
                        gidx_t[:, s0 // 16 : (s0 + n_idx) // 16],
                        n_idx, n_idx, P, single_packet=False)
                    ohblk = ohp.tile([P, GCALL // P * WIN], BF16, tag="ohblk")
                    nc.sync.dma_start(ohblk[:, : ncall * WIN],
                                      ohtab[:, c0 * WIN : (c0 + ncall) * WIN])
                    for cc in range(ncall):
                        ch = c0 + cc
                        ri, j, nchk = ch_run[ch]
                        w, sh = runs[ri]
                        if j == 0:
                            ps = ps_run.tile([P, WIN], f32, tag="psw")
                        nc.tensor.matmul(ps[:], lhsT=g[:, cc, :],
                                         rhs=ohblk[:, cc * WIN : (cc + 1) * WIN],
                                         start=(j == 0), stop=(j == nchk - 1))
                        if j == nchk - 1:
                            wsl = aggr[:, w * WIN : (w + 1) * WIN]
                            first_sh = 1 if run_nch[runs.index((w, 1))] else 0
                            last_sh = 0 if run_nch[runs.index((w, 0))] else 1
                            if sh == first_sh:
                                nc.scalar.activation(wsl, ps[:], AF.Copy)
                            else:
                                nc.vector.tensor_tensor(wsl, wsl, ps[:], ALU.add)
                            if sh == last_sh:
                                win_tail(layer, curT, nxtT, w)
                    for gg in range(NCHK):
                        if agcall[gg] == ci:
                            emit_ag(layer, (gg + 4) % NCHK)
                curT, nxtT = nxtT, curT

            # ---- candidate gathers (transposed: [feat, slot] columns) ----
            ut = bp.tile([P, 1, CSLOT], BF16, tag="aggr")
            vt = bp.tile([P, 1, CSLOT], BF16, tag="curT")
            for (tl, idx_t, ccalls, cofs) in ((ut, cu_t, cm["ucalls"], 0),
                                              (vt, cv_t, cm["vcalls"], P)):
                for (s0, n_idx, s) in ccalls:
                    nc.gpsimd.dma_gather(
                        tl[:, :, s0 : s0 + n_idx],
                        gqfull[s * LOB : NTOT if s else LOB, cofs : cofs + P],
                        idx_t[:, s0 // 16 : (s0 + n_idx) // 16],
                        n_idx, n_idx, P, elem_step=2 * P, transpose=True,
                        single_packet=False)
            # ---- candidate MLP (slot tiles of 512 columns) ----
            ycol = wp.tile([P, NCC], f32, tag="ycol")
            for s0 in range(0, CSLOT, 512):
                W = min(512, CSLOT - s0)
                pz1 = ps_s.tile([64, 512], f32, tag="pz1")
                nc.tensor.matmul(pz1[:, :W], lhsT=w0r1_t[:],
                                 rhs=featrow_t[:, s0 : s0 + W],
                                 start=True, stop=False)
                nc.tensor.matmul(pz1[:, :W], lhsT=i64_t[:],
                                 rhs=ut[0:64, 0, s0 : s0 + W],
                                 start=False, stop=False)
                nc.tensor.matmul(pz1[:, :W], lhsT=i64_t[:],
                                 rhs=vt[0:64, 0, s0 : s0 + W],
                                 start=False, stop=True)
                z1f = wp.tile([64, 512], BF16, tag="z1f")
                nc.scalar.activation(z1f[:, :W], pz1[:, :W], AF.Lrelu, alpha=SLOPE)
                pz2 = ps_s.tile([64, 512], f32, tag="pz2")
                nc.tensor.matmul(pz2[:, :W], lhsT=mw1b_t[:], rhs=z1f[:, :W],
                                 start=True, stop=False)
                nc.tensor.matmul(pz2[:, :W], lhsT=mb1r1_t[:],
                                 rhs=ones512_t[:, :W], start=False, stop=True)
                z2f = wp.tile([64, 512], BF16, tag="z2f")
                nc.scalar.activation(z2f[:, :W], pz2[:, :W], AF.Lrelu, alpha=SLOPE)
                py = ps_run.tile([P, 4], f32, tag="psw")
                for gsub in range(W // P):
                    nc.tensor.matmul(py[:, gsub : gsub + 1],
                                     lhsT=z2f[:, gsub * P : (gsub + 1) * P],
                                     rhs=mw2c_t[:], start=True, stop=True)
                nc.scalar.activation(ycol[:, s0 // P : s0 // P + W // P],
                                     py[:, : W // P], AF.Identity,
                                     bias=mb2_t[:, 0:1])
            nc.sync.dma_start(y_out[:], ycol[:])
            ym = wp.tile([P, NCC], f32, tag="ym")
            nc.vector.tensor_tensor(ym[:], ycol[:], mask_t[:], ALU.add)
            nc.sync.dma_start(ysh[:], ym[:])
            nc.gpsimd.collective_compute(
                "AllGather", ALU.bypass, replica_groups=rg,
                ins=[ysh[:].opt()], outs=[yfull[:].opt()])
            # ---- softmax ----
            ncols = NCORE * CSLOT // P
            yf = bp.tile([P, ncols], f32, tag="yf")
            nc.sync.dma_start(yf[:], yfull[:].rearrange("a b -> (a b)")
                              .rearrange("(p c) -> p c", p=P))
            rmax = wp.tile([P, 1], f32, tag="rmax")
            nc.vector.tensor_reduce(rmax[:], yf[:], mybir.AxisListType.X, ALU.max)
            gmax = wp.tile([P, 1], f32, tag="gmax")
            nc.gpsimd.partition_all_reduce(gmax[:], rmax[:], P,
                                           bass_isa.ReduceOp.max)
            ngmax = wp.tile([P, 1], f32, tag="ngmax")
            nc.vector.tensor_scalar(ngmax[:], gmax[:], -1.0, None, ALU.mult)
            ef = bp.tile([P, ncols], f32, tag="ef")
            se = wp.tile([P, 1], f32, tag="se")
            nc.scalar.activation(ef[:], yf[:], AF.Exp, bias=ngmax[:, 0:1],
                                 accum_out=se[:])
            stot = wp.tile([P, 1], f32, tag="stot")
            nc.gpsimd.partition_all_reduce(stot[:], se[:], P, bass_isa.ReduceOp.add)
            invs = wp.tile([P, 1], f32, tag="invs")
            nc.vector.reciprocal(invs[:], stot[:])
            pf = bp.tile([P, ncols], f32, tag="pf")
            nc.vector.tensor_scalar(pf[:], ef[:], invs[:, 0:1], None, ALU.mult)
            nc.sync.dma_start(p_out[:], pf[:])
    nc.compile()
    return nc


def kernel(x, src, dst, cand_u, cand_v, cand_feat,
           w_self0, w_neigh0, b0, gamma0, beta0, rm0, rv0,
           w_self1, w_neigh1, b1, gamma1, beta1, rm1, rv1,
           mw0, mb0, mw1, mb1, mw2, mb2):
    x = np.asarray(x, np.float32)
    src = np.asarray(src, np.int64)
    dst = np.asarray(dst, np.int64)
    cand_u = np.asarray(cand_u, np.int64)
    cand_v = np.asarray(cand_v, np.int64)
    cand_feat = np.asarray(cand_feat, np.float32)

    deg = np.bincount(dst, minlength=N).astype(np.float32)
    invdeg = 1.0 / np.maximum(deg, 1.0)
    em, edata = _prep_edges(src, dst, invdeg)
    cm, cdata = _prep_cands(cand_u, cand_v, cand_feat)

    xpad = np.zeros((NTOT, D), np.float32)
    xpad[:N] = x
    xperm = np.zeros((NTOT, D), np.float32)
    xperm[_newrow(np.arange(NTOT))] = xpad
    ident = np.eye(P, dtype=np.float32)
    onesr = np.ones((1, P), np.float32)

    com = {"xpad": xperm.astype(NPBF), "ident": ident, "onesr": onesr}
    for l, (ws, wn, b, ga, be, rme, rve) in enumerate(
        ((w_self0, w_neigh0, b0, gamma0, beta0, rm0, rv0),
         (w_self1, w_neigh1, b1, gamma1, beta1, rm1, rv1))):
        a = (ga / np.sqrt(rve + BN_EPS)).astype(np.float32)
        com[f"wself{l}"] = (ws * a[None, :]).astype(np.float32)
        com[f"wneigh{l}"] = (wn * a[None, :]).astype(np.float32)
        com[f"crow{l}"] = (a * (b - rme) + be).astype(np.float32)[None, :]
    com["amat"] = np.asarray(mw0[0:128], np.float32)
    com["bmat"] = np.asarray(mw0[128:256], np.float32)
    com["mb0r"] = np.asarray(mb0, np.float32)[None, :]
    com["w0r1"] = np.asarray(mw0[256], np.float32)[None, :].astype(NPBF)
    com["i64"] = np.eye(64, dtype=np.float32).astype(NPBF)
    com["mw1b"] = np.asarray(mw1, np.float32).astype(NPBF)
    com["mb1r1"] = np.asarray(mb1, np.float32)[None, :].astype(NPBF)
    com["mw2c"] = np.asarray(mw2, np.float32).astype(NPBF)
    com["mb2v"] = np.full((P, 1), np.float32(np.asarray(mb2).ravel()[0]))
    com["ones512"] = np.ones((1, 512), np.float32).astype(NPBF)

    nc = _build_nc(em, cm)
    in_maps = []
    for k in range(NCORE):
        m = dict(com)
        m["xT"] = xpad[k * NSH : (k + 1) * NSH].T.copy()
        m["gidx"] = edata[k]["gidx"]
        m["ohtab"] = edata[k]["ohtab"]
        m["cu"] = cdata[k]["cu"]
        m["cv"] = cdata[k]["cv"]
        m["featrow"] = cdata[k]["featrow"]
        m["maskr"] = cdata[k]["mask"]
        in_maps.append(m)
    import os
    trace = bool(os.environ.get("KERNEL_TRACE"))
    if trace:
        import types
        import ctypes
        if "antenv.axon_hooks" not in sys.modules:
            try:
                import antenv
                from trn_agent_boot.trn_boot import _ntff_profile_via_ctypes
                mod = types.ModuleType("antenv.axon_hooks")
                hook = [_ntff_profile_via_ctypes("/opt/axon/libaxon_pjrt.so")]
                mod.set_axon_ntff_profile_hook = lambda h: hook.__setitem__(0, h)
                mod.get_axon_ntff_profile_hook = lambda: hook[0]
                sys.modules["antenv.axon_hooks"] = mod
                antenv.axon_hooks = mod
            except Exception:
                trace = False
    res = run_bass_kernel_spmd(nc, in_maps, core_ids=list(range(NCORE)),
                               trace=trace,
                               tmpdir=os.environ.get("KERNEL_TRACE_DIR"))
    if trace and res.exec_time_ns is not None:
        print(f"HW exec time: {res.exec_time_ns} ns")
    y_all = np.zeros(C, np.float32)
    p_all = np.zeros(C, np.float32)
    ncc = cm["ncc"]
    p_lin = res.results[0]["p_out"].ravel()   # global order: k, p, c
    for k in range(NCORE):
        sm = cdata[k]["slotmap"]
        valid = sm >= 0
        j = np.nonzero(valid)[0]              # slot j = c*128 + p
        yk = res.results[k]["y_out"]          # [128, NCC] -> value at [j%128, j//128]
        y_all[sm[valid]] = yk[j % P, j // P]
        gs = k * cm["cslot"] + (j % P) * ncc + (j // P)
        p_all[sm[valid]] = p_lin[gs]
    return y_all[:, None], p_all[:, None]


# revision 35
# speedup vs baseline: 1.0283x; 1.0283x over previous
"""Trainium2 Bass kernel for nn_PolicyNetwork3 (2-layer GraphSAGE + edge-MLP).

Design (8 NeuronCores, SPMD single NEFF):
- dst-sharded aggregation: core k owns node block [6272k, 6272k+6272).
- Edges sorted by (core, dst-window of 128, src-half); messages gathered from
  bf16 HBM row tables via dma_gather (int16 idx; uneven LO/HI table split at
  row 28672 so both halves fit int16 and align to AllGather chunks).
- segment-sum per 128-dst window via matmuls against host-precomputed bf16
  invdeg-one-hot blocks streamed from HBM; PSUM accumulation per window.
- BN folded into the SAGE weights on host; leaky-relu on the Scalar engine.
- Global node tables (xpad/hfull/gqfull) use a chunk-interleaved row layout:
  row(n) = (l//896)*7168 + core*896 + (l%896), so the h/gq shard exchange runs
  as 7 chunked AllGathers that overlap the gather stream; only the last chunk
  sits on the critical path.
- candidate MLP folds its first layer into per-node tables [g|q|q|g]; u/v rows
  are fetched with transposed gathers ([feat, slot] layout) and the MLP runs
  as wide PE matmuls over 512-slot tiles; global softmax after an AllGather.
"""

import sys

sys.path.insert(0, "/opt/trn_rl_repo")
sys.path.insert(0, "/root/.axon_site")

import ml_dtypes
import numpy as np

import concourse.bacc as bacc
import concourse.bass as bass
import concourse.bass_isa as bass_isa
import concourse.mybir as mybir
import concourse.tile as tile
from concourse import library_config
from concourse.bass_utils import run_bass_kernel_spmd

P = 128
N, E, C = 50000, 800000, 100000
D = 128
NCORE = 8
NSH = 6272          # nodes per core shard
NTOT = NSH * NCORE  # 50176 padded node table
WIN = 128           # dst nodes per aggregation window
NWIN = NSH // WIN   # 49 windows == linear blocks per core
NBLK = NWIN
NCHK = 7            # AllGather chunks per shard
CJ = NSH // NCHK    # 896 rows per core per chunk
GJ = NCORE * CJ     # 7168 global rows per chunk
LOB = 4 * GJ        # 28672: LO/HI gather-table split (both halves < 2^15)
CSH = C // NCORE    # 12500 candidates per core
GCALL = 4096        # max idxs per dma_gather call
AGSLACK = 3         # extra gather calls before an AG trigger
BN_EPS = 1e-5
SLOPE = 0.01
F32 = mybir.dt.float32
BF16 = mybir.dt.bfloat16
I16 = mybir.dt.int16
AF = mybir.ActivationFunctionType
ALU = mybir.AluOpType
NPBF = ml_dtypes.bfloat16


def _newrow(n):
    """Original padded node id -> chunk-interleaved global table row.

    Window-group g (windows 7g..7g+6, completing g-th in stream order) feeds
    AG chunk (g+4)%7, so the HI half of the table (chunks 4-6) is ready
    earliest -- the next layer's gather stream starts with HI calls."""
    k = n // NSH
    l = n - k * NSH
    return ((l // CJ + 4) % NCHK) * GJ + k * CJ + (l % CJ)


def _wrap16(idx_lin):
    """[n] -> [128, n/16] int16 in the dma_gather wrapped+replicated layout."""
    n = idx_lin.shape[0]
    assert n % 16 == 0
    w = idx_lin.reshape(n // 16, 16).T.astype(np.int16)
    return np.tile(w, (8, 1)).copy()


def gidx_to_cols(arr):
    """[nslot] -> [128, nchunk] with slot i at [i%128, i//128]."""
    n = arr.shape[0]
    return arr.reshape(n // P, P).T.copy()


def _prep_edges(src, dst, invdeg):
    """Build the uniform per-core chunk schedule + per-core index data."""
    core = dst // NSH
    dl = dst - core * NSH
    winl = dl // WIN
    srow = _newrow(src)
    half = (srow >= LOB).astype(np.int64)
    key = (core * NWIN + winl) * 2 + half
    order = np.argsort(key, kind="stable")
    cnt = np.bincount(key, minlength=NCORE * NWIN * 2).reshape(NCORE, NWIN * 2)
    nch = -(-cnt // P)                       # ceil chunks per (core, win*2+half)
    nch_u = nch.max(axis=0)                  # [NWIN*2] uniform chunk counts
    # stream order: all HI runs (win 0..48), then all LO runs; windows then
    # complete spread across the longer LO pass, spacing the AG triggers
    runs = [(w, s) for s in (1, 0) for w in range(NWIN)]
    run_nch = [int(nch_u[w * 2 + s]) for (w, s) in runs]
    tot_ch = sum(run_nch)
    nslot = tot_ch * P

    gidx = np.zeros((NCORE, nslot), np.int16)
    dstloc = np.full((NCORE, nslot), -5, np.int64)
    val = np.zeros((NCORE, nslot), np.float32)
    bstart = np.zeros(NCORE * NWIN * 2 + 1, np.int64)
    np.cumsum(np.bincount(key, minlength=NCORE * NWIN * 2), out=bstart[1:])
    for k in range(NCORE):
        pos = 0
        for (w, s) in runs:
            b = (k * NWIN + w) * 2 + s
            e0, e1 = bstart[b], bstart[b + 1]
            n = e1 - e0
            sl = order[e0:e1]
            sl = sl[np.argsort(srow[sl], kind="stable")]
            gidx[k, pos : pos + n] = (srow[sl] - s * LOB).astype(np.int16)
            dstloc[k, pos : pos + n] = dl[sl] % WIN
            val[k, pos : pos + n] = invdeg[dst[sl]]
            pos += nch_u[w * 2 + s] * P
    # gather call split: contiguous HI slots then LO slots, calls <= GCALL
    n_hi = sum(run_nch[:NWIN]) * P
    calls = []  # (slot_start, n_idx, half)
    for s, lo, hi in ((1, 0, n_hi), (0, n_hi, nslot)):
        p = lo
        while p < hi:
            n = min(GCALL, hi - p)
            calls.append((p, n, s))
            p += n
    # per-run chunk offsets
    run_off = np.zeros(len(runs) + 1, np.int64)
    np.cumsum(run_nch, out=run_off[1:])
    # AG trigger schedule: AG chunk q needs windows 7q..7q+6 complete (their
    # HI runs); trigger after the gather call covering that slot, plus slack.
    agcall = []
    for q in range(NCHK):
        end_slot = run_off[NWIN + 7 * q + 7] * P  # end of HI run of win 7q+6
        ci = next(i for i, (s0, n, s) in enumerate(calls) if s0 + n >= end_slot)
        agcall.append(min(ci + (5 if q <= 4 else AGSLACK), len(calls) - 1))
    meta = dict(runs=runs, run_nch=run_nch, run_off=run_off, tot_ch=tot_ch,
                nslot=nslot, calls=calls, agcall=agcall)
    data = []
    for k in range(NCORE):
        # one-hot scatter table: slot i = chunk i//128, partition i%128
        oh = np.zeros((tot_ch, P, WIN), np.float32)
        sl = np.nonzero(dstloc[k] >= 0)[0]
        oh[sl // P, sl % P, dstloc[k][sl]] = val[k][sl]
        data.append(dict(gidx=_wrap16(gidx[k]),
                         ohtab=oh.transpose(1, 0, 2).reshape(P, tot_ch * WIN)
                         .astype(NPBF)))
    return meta, data


def _prep_cands(cand_u, cand_v, cand_feat):
    """Shard candidates, group by (u_half, v_half), pad to uniform chunks."""
    urow = _newrow(cand_u)
    vrow = _newrow(cand_v)
    percore = [np.arange(k * CSH, (k + 1) * CSH) for k in range(NCORE)]
    groups = [[None] * 4 for _ in range(NCORE)]
    for k in range(NCORE):
        ids = percore[k]
        g = (urow[ids] >= LOB) * 2 + (vrow[ids] >= LOB)
        o = np.argsort(g, kind="stable")
        ids = ids[o]
        gs = g[o]
        for gi in range(4):
            gids = ids[gs == gi]
            groups[k][gi] = gids[np.argsort(urow[gids], kind="stable")]
    gch = np.zeros((NCORE, 4), np.int64)
    for k in range(NCORE):
        for gi in range(4):
            gch[k, gi] = -(-len(groups[k][gi]) // P)
    gorder = [3, 2, 1, 0]
    gch_u = gch.max(axis=0)                 # uniform chunks per group
    ncc = int(gch_u.sum())
    cslot = ncc * P
    cu = np.zeros((NCORE, cslot), np.int16)
    cv = np.zeros((NCORE, cslot), np.int16)
    ft = np.zeros((NCORE, cslot), np.float32)
    mask = np.full((NCORE, cslot), -1e30, np.float32)
    slotmap = np.full((NCORE, cslot), -1, np.int64)
    goff = np.zeros(5, np.int64)
    np.cumsum(gch_u[gorder] * P, out=goff[1:])
    gpos = {gi: goff[i] for i, gi in enumerate(gorder)}
    for k in range(NCORE):
        for gi in range(4):
            ids = groups[k][gi]
            n = len(ids)
            p0 = gpos[gi]
            uh, vh = gi // 2, gi % 2
            cu[k, p0 : p0 + n] = (urow[ids] - uh * LOB).astype(np.int16)
            cv[k, p0 : p0 + n] = (vrow[ids] - vh * LOB).astype(np.int16)
            ft[k, p0 : p0 + n] = cand_feat[ids, 0]
            mask[k, p0 : p0 + n] = 0.0
            slotmap[k, p0 : p0 + n] = ids
    # gather calls: u -> runs (groups 3-2 HI | 1-0 LO); v -> one run per group
    ucalls, vcalls = [], []
    for s, lo, hi in ((1, goff[0], goff[2]), (0, goff[2], goff[4])):
        p = lo
        while p < hi:
            n = min(GCALL, hi - p)
            ucalls.append((int(p), int(n), s))
            p += n
    for i, gi in enumerate(gorder):
        p, hi = goff[i], goff[i + 1]
        while p < hi:
            n = min(GCALL, hi - p)
            vcalls.append((int(p), int(n), gi % 2))
            p += n
    meta = dict(ncc=ncc, cslot=cslot, ucalls=ucalls, vcalls=vcalls)
    data = [dict(cu=_wrap16(cu[k]), cv=_wrap16(cv[k]),
                 featrow=ft[k][None, :].astype(NPBF),
                 mask=gidx_to_cols(mask[k]),
                 slotmap=slotmap[k]) for k in range(NCORE)]
    return meta, data


def _build_nc(em, cm):
    nc = bacc.Bacc("TRN2", target_bir_lowering=False, debug=False,
                   num_devices=NCORE)
    f32 = F32
    TOTCH, NSLOT = em["tot_ch"], em["nslot"]
    NCC, CSLOT = cm["ncc"], cm["cslot"]

    # ---- external inputs ----
    xpad = nc.dram_tensor("xpad", [NTOT, D], BF16, kind="ExternalInput")
    xT = nc.dram_tensor("xT", [P, NSH], f32, kind="ExternalInput")
    gidx = nc.dram_tensor("gidx", [P, NSLOT // 16], I16, kind="ExternalInput")
    ohtab = nc.dram_tensor("ohtab", [P, TOTCH * WIN], BF16, kind="ExternalInput")
    wself = [nc.dram_tensor(f"wself{l}", [D, D], f32, kind="ExternalInput") for l in range(2)]
    wneigh = [nc.dram_tensor(f"wneigh{l}", [D, D], f32, kind="ExternalInput") for l in range(2)]
    crow = [nc.dram_tensor(f"crow{l}", [1, D], f32, kind="ExternalInput") for l in range(2)]
    ident = nc.dram_tensor("ident", [P, P], f32, kind="ExternalInput")
    onesr = nc.dram_tensor("onesr", [1, P], f32, kind="ExternalInput")
    amat = nc.dram_tensor("amat", [D, 64], f32, kind="ExternalInput")
    bmat = nc.dram_tensor("bmat", [D, 64], f32, kind="ExternalInput")
    mb0r = nc.dram_tensor("mb0r", [1, 64], f32, kind="ExternalInput")
    w0r1 = nc.dram_tensor("w0r1", [1, 64], BF16, kind="ExternalInput")
    i64 = nc.dram_tensor("i64", [64, 64], BF16, kind="ExternalInput")
    mw1b = nc.dram_tensor("mw1b", [64, 64], BF16, kind="ExternalInput")
    mb1r1 = nc.dram_tensor("mb1r1", [1, 64], BF16, kind="ExternalInput")
    mw2c = nc.dram_tensor("mw2c", [64, 1], BF16, kind="ExternalInput")
    mb2v = nc.dram_tensor("mb2v", [P, 1], f32, kind="ExternalInput")
    ones512 = nc.dram_tensor("ones512", [1, 512], BF16, kind="ExternalInput")
    cu = nc.dram_tensor("cu", [P, CSLOT // 16], I16, kind="ExternalInput")
    cv = nc.dram_tensor("cv", [P, CSLOT // 16], I16, kind="ExternalInput")
    featrow = nc.dram_tensor("featrow", [1, CSLOT], BF16, kind="ExternalInput")
    maskr = nc.dram_tensor("maskr", [P, NCC], f32, kind="ExternalInput")
    # ---- outputs ----
    y_out = nc.dram_tensor("y_out", [P, NCC], f32, kind="ExternalOutput")
    p_out = nc.dram_tensor("p_out", [P, NCORE * CSLOT // P], f32, kind="ExternalOutput")
    # ---- internal DRAM ----
    hshq = [nc.dram_tensor(f"hshq{q}", [CJ, D], BF16, kind="Internal")
            for q in range(NCHK)]
    hfull = nc.dram_tensor("hfull", [NTOT, D], BF16, kind="Internal", addr_space="Shared")
    gqshq = [nc.dram_tensor(f"gqshq{q}", [CJ, 2 * D], BF16, kind="Internal")
             for q in range(NCHK)]
    gqfull = nc.dram_tensor("gqfull", [NTOT, 2 * D], BF16, kind="Internal", addr_space="Shared")
    ysh = nc.dram_tensor("ysh", [P, NCC], f32, kind="Internal")
    yfull = nc.dram_tensor("yfull", [NCORE * P, NCC], f32, kind="Internal", addr_space="Shared")

    rg = [list(range(NCORE))]

    with tile.TileContext(nc) as tc:
        with (
            tc.tile_pool(name="const", bufs=1) as cp,
            tc.tile_pool(name="big", bufs=1) as bp,
            tc.tile_pool(name="msgs", bufs=3) as mp,
            tc.tile_pool(name="oh", bufs=3) as ohp,
            tc.tile_pool(name="wrk", bufs=4) as wp,
            tc.tile_pool(name="ps_run", bufs=2, space="PSUM") as ps_run,
            tc.tile_pool(name="ps_t", bufs=2, space="PSUM") as ps_t,
            tc.tile_pool(name="ps_h", bufs=2, space="PSUM") as ps_h,
            tc.tile_pool(name="ps_s", bufs=1, space="PSUM") as ps_s,
        ):
            nc.gpsimd.load_library(library_config.mlp)

            def load(pool, t, shape=None):
                tl = pool.tile(shape or list(t.shape), t.dtype, tag=t.name)
                nc.sync.dma_start(tl[:], t[:])
                return tl

            gidx_t = load(cp, gidx)
            ident_t = load(cp, ident)
            onesr_t = load(cp, onesr)
            wself_t = [load(cp, w) for w in wself]
            wneigh_t = [load(cp, w) for w in wneigh]
            crow_t = [load(cp, w) for w in crow]
            amat_t = load(cp, amat)
            bmat_t = load(cp, bmat)
            mb0r_t = load(cp, mb0r)
            w0r1_t = load(cp, w0r1)
            i64_t = load(cp, i64)
            mw1b_t = load(cp, mw1b)
            mb1r1_t = load(cp, mb1r1)
            mw2c_t = load(cp, mw2c)
            mb2_t = load(cp, mb2v)
            ones512_t = load(cp, ones512)
            cu_t = load(cp, cu)
            cv_t = load(cp, cv)
            featrow_t = load(cp, featrow)
            mask_t = load(cp, maskr)

            curT = bp.tile([P, NSH], f32, tag="curT")
            nxtT = bp.tile([P, NSH], f32, tag="nxtT")
            aggr = bp.tile([P, NSH], f32, tag="aggr")
            nc.sync.dma_start(curT[:], xT[:])

            runs, run_nch, run_off = em["runs"], em["run_nch"], em["run_off"]
            calls, agcall = em["calls"], em["agcall"]
            # chunk -> (run_index, pos_in_run)
            ch_run = []
            for ri, nchk in enumerate(run_nch):
                ch_run += [(ri, j, nchk) for j in range(nchk)]
            pending_ag = []

            def win_tail(layer, cT, nT, w):
                """Linear block + h/gq table writes once window w is final."""
                ph = ps_h.tile([P, P], f32, tag="ph")
                nc.tensor.matmul(ph[:], lhsT=aggr[:, w * P : (w + 1) * P],
                                 rhs=wneigh_t[layer][:],
                                 start=True, stop=False)
                nc.tensor.matmul(ph[:], lhsT=cT[:, w * P : (w + 1) * P],
                                 rhs=wself_t[layer][:], start=False, stop=False)
                nc.tensor.matmul(ph[:], lhsT=onesr_t[:], rhs=crow_t[layer][:],
                                 start=False, stop=True)
                ht = wp.tile([P, P], f32, tag="ht")
                nc.scalar.activation(ht[:], ph[:], AF.Lrelu, alpha=SLOPE)
                q, r = (w // NCHK + 4) % NCHK, (w % NCHK) * P
                if layer == 0:
                    htb = wp.tile([P, P], BF16, tag="htb")
                    nc.scalar.activation(htb[:], ph[:], AF.Lrelu, alpha=SLOPE)
                    nc.sync.dma_start(hshq[q][r : r + P, :], htb[:])
                pt2 = ps_t.tile([P, P], f32, tag="pt")
                nc.tensor.transpose(pt2[:], ht[:], ident_t[:])
                nc.scalar.activation(nT[:, w * P : (w + 1) * P], pt2[:], AF.Copy)
                if layer == 1:
                    # g/q tables ([g|q] cols 0:128, [q|g] cols 128:256)
                    h2T = nT[:, w * P : (w + 1) * P]
                    pg = ps_h.tile([P, 64], f32, tag="ph")
                    nc.tensor.matmul(pg[:], lhsT=h2T, rhs=amat_t[:],
                                     start=True, stop=True)
                    pq = ps_t.tile([P, 64], f32, tag="pt")
                    nc.tensor.matmul(pq[:], lhsT=h2T, rhs=bmat_t[:],
                                     start=True, stop=False)
                    nc.tensor.matmul(pq[:], lhsT=onesr_t[:], rhs=mb0r_t[:],
                                     start=False, stop=True)
                    gq = wp.tile([P, 2 * P], BF16, tag="gq")
                    nc.scalar.activation(gq[:, 0:64], pg[:], AF.Copy)
                    nc.scalar.activation(gq[:, 64:128], pq[:], AF.Copy)
                    nc.scalar.activation(gq[:, 128:192], pq[:], AF.Copy)
                    nc.scalar.activation(gq[:, 192:256], pg[:], AF.Copy)
                    nc.sync.dma_start(gqshq[q][r : r + P, :], gq[:])

            def emit_ag(layer, q):
                if layer == 0:
                    nc.gpsimd.collective_compute(
                        "AllGather", ALU.bypass, replica_groups=rg,
                        ins=[hshq[q][:].opt()],
                        outs=[hfull[q * GJ : (q + 1) * GJ, :].opt()])
                else:
                    nc.gpsimd.collective_compute(
                        "AllGather", ALU.bypass, replica_groups=rg,
                        ins=[gqshq[q][:].opt()],
                        outs=[gqfull[q * GJ : (q + 1) * GJ, :].opt()])

            for layer in range(2):
                gtab = xpad if layer == 0 else hfull
                ps = None
                for ci, (s0, n_idx, s) in enumerate(calls):
                    c0, ncall = s0 // P, n_idx // P
                    g = mp.tile([P, GCALL // P, P], BF16, tag="g")
                    nc.gpsimd.dma_gather(
                        g[:, :ncall, :],
                        gtab[s * LOB : NTOT if s else LOB, :],
                        gidx_t[:, s0 // 16 : (s0 + n_idx) // 16],
                        n_idx, n_idx, P, single_packet=False)
                    ohblk = ohp.tile([P, GCALL // P * WIN], BF16, tag="ohblk")
                    nc.sync.dma_start(ohblk[:, : ncall * WIN],
                                      ohtab[:, c0 * WIN : (c0 + ncall) * WIN])
                    for cc in range(ncall):
                        ch = c0 + cc
                        ri, j, nchk = ch_run[ch]
                        w, sh = runs[ri]
                        if j == 0:
                            ps = ps_run.tile([P, WIN], f32, tag="psw")
                        nc.tensor.matmul(ps[:], lhsT=g[:, cc, :],
                                         rhs=ohblk[:, cc * WIN : (cc + 1) * WIN],
                                         start=(j == 0), stop=(j == nchk - 1))
                        if j == nchk - 1:
                            wsl = aggr[:, w * WIN : (w + 1) * WIN]
                            first_sh = 1 if run_nch[runs.index((w, 1))] else 0
                            last_sh = 0 if run_nch[runs.index((w, 0))] else 1
                            if sh == first_sh:
                                nc.scalar.activation(wsl, ps[:], AF.Copy)
                            else:
                                nc.vector.tensor_tensor(wsl, wsl, ps[:], ALU.add)
                            if sh == last_sh:
                                win_tail(layer, curT, nxtT, w)
                    for gg in range(NCHK):
                        if agcall[gg] == ci:
                            if layer == 1 and ci == len(calls) - 1:
                                pending_ag.append((gg + 4) % NCHK)
                            else:
                                emit_ag(layer, (gg + 4) % NCHK)
                curT, nxtT = nxtT, curT

            # ---- candidate phase: gathers + MLP interleaved in slot order.
            # Per-call tiles recycle the edge-phase pools (ohblk/g tags, same
            # 8KB footprint); each call's MLP tiles are emitted before the
            # next same-tag call, so pool rotation can never clobber live
            # data, and the MLP overlaps the gather stream on PE/ACT.
            ucl = list(cm["ucalls"])
            vcl = list(cm["vcalls"])
            uticur = [None]
            vticur = [None]
            nemit = [0]

            def cgather(uv, call):
                s0, n_idx, s = call
                pool, tag, idx_t, cofs = ((ohp, "ohblk", cu_t, 0) if uv == "u"
                                          else (mp, "g", cv_t, P))
                tl = pool.tile([P, 1, GCALL], BF16, tag=tag)
                nc.gpsimd.dma_gather(
                    tl[:, :, :n_idx],
                    gqfull[s * LOB : NTOT if s else LOB, cofs : cofs + P],
                    idx_t[:, s0 // 16 : (s0 + n_idx) // 16],
                    n_idx, n_idx, P, elem_step=2 * P, transpose=True,
                    single_packet=False)
                nemit[0] += 1
                if nemit[0] == 1:
                    for q in pending_ag:
                        emit_ag(1, q)
                    pending_ag.clear()
                return (s0, n_idx, tl)

            def cslice(cur, cl, uv, a, b):
                while cur[0] is None or a >= cur[0][0] + cur[0][1]:
                    cur[0] = cgather(uv, cl.pop(0))
                s0, n_idx, tl = cur[0]
                assert s0 <= a and b <= s0 + n_idx, (uv, a, b, s0, n_idx)
                return tl[0:64, 0, a - s0 : b - s0]

            cuts = sorted({c for c in range(0, CSLOT, 512)}
                          | {s0 for (s0, _, _) in ucl}
                          | {s0 for (s0, _, _) in vcl} | {CSLOT})
            ycol = wp.tile([P, NCC], f32, tag="ycol")
            for a, b in zip(cuts[:-1], cuts[1:]):
                W = b - a
                urhs = cslice(uticur, ucl, "u", a, b)
                vrhs = cslice(vticur, vcl, "v", a, b)
                pz1 = ps_s.tile([64, 512], f32, tag="pz1")
                nc.tensor.matmul(pz1[:, :W], lhsT=w0r1_t[:],
                                 rhs=featrow_t[:, a : b],
                                 start=True, stop=False)
                nc.tensor.matmul(pz1[:, :W], lhsT=i64_t[:],
                                 rhs=urhs, start=False, stop=False)
                nc.tensor.matmul(pz1[:, :W], lhsT=i64_t[:],
                                 rhs=vrhs, start=False, stop=True)
                z1f = wp.tile([64, 512], BF16, tag="z1f")
                nc.scalar.activation(z1f[:, :W], pz1[:, :W], AF.Lrelu, alpha=SLOPE)
                pz2 = ps_s.tile([64, 512], f32, tag="pz2")
                nc.tensor.matmul(pz2[:, :W], lhsT=mw1b_t[:], rhs=z1f[:, :W],
                                 start=True, stop=False)
                nc.tensor.matmul(pz2[:, :W], lhsT=mb1r1_t[:],
                                 rhs=ones512_t[:, :W], start=False, stop=True)
                z2f = wp.tile([64, 512], BF16, tag="z2f")
                nc.scalar.activation(z2f[:, :W], pz2[:, :W], AF.Lrelu, alpha=SLOPE)
                py = ps_run.tile([P, 4], f32, tag="psw")
                for gsub in range(W // P):
                    nc.tensor.matmul(py[:, gsub : gsub + 1],
                                     lhsT=z2f[:, gsub * P : (gsub + 1) * P],
                                     rhs=mw2c_t[:], start=True, stop=True)
                nc.scalar.activation(ycol[:, a // P : b // P],
                                     py[:, : W // P], AF.Identity,
                                     bias=mb2_t[:, 0:1])
            nc.sync.dma_start(y_out[:], ycol[:])
            ym = wp.tile([P, NCC], f32, tag="ym")
            nc.vector.tensor_tensor(ym[:], ycol[:], mask_t[:], ALU.add)
            nc.sync.dma_start(ysh[:], ym[:])
            nc.gpsimd.collective_compute(
                "AllGather", ALU.bypass, replica_groups=rg,
                ins=[ysh[:].opt()], outs=[yfull[:].opt()])
            # ---- softmax ----
            ncols = NCORE * CSLOT // P
            yf = bp.tile([P, ncols], f32, tag="yf")
            nc.sync.dma_start(yf[:], yfull[:].rearrange("a b -> (a b)")
                              .rearrange("(p c) -> p c", p=P))
            rmax = wp.tile([P, 1], f32, tag="rmax")
            nc.vector.tensor_reduce(rmax[:], yf[:], mybir.AxisListType.X, ALU.max)
            gmax = wp.tile([P, 1], f32, tag="gmax")
            nc.gpsimd.partition_all_reduce(gmax[:], rmax[:], P,
                                           bass_isa.ReduceOp.max)
            ngmax = wp.tile([P, 1], f32, tag="ngmax")
            nc.vector.tensor_scalar(ngmax[:], gmax[:], -1.0, None, ALU.mult)
            ef = bp.tile([P, ncols], f32, tag="ef")
            se = wp.tile([P, 1], f32, tag="se")
            nc.scalar.activation(ef[:], yf[:], AF.Exp, bias=ngmax[:, 0:1],
                                 accum_out=se[:])
            stot = wp.tile([P, 1], f32, tag="stot")
            nc.gpsimd.partition_all_reduce(stot[:], se[:], P, bass_isa.ReduceOp.add)
            invs = wp.tile([P, 1], f32, tag="invs")
            nc.vector.reciprocal(invs[:], stot[:])
            pf = bp.tile([P, ncols], f32, tag="pf")
            nc.vector.tensor_scalar(pf[:], ef[:], invs[:, 0:1], None, ALU.mult)
            nc.sync.dma_start(p_out[:], pf[:])
    nc.compile()
    return nc


def kernel(x, src, dst, cand_u, cand_v, cand_feat,
           w_self0, w_neigh0, b0, gamma0, beta0, rm0, rv0,
           w_self1, w_neigh1, b1, gamma1, beta1, rm1, rv1,
           mw0, mb0, mw1, mb1, mw2, mb2):
    x = np.asarray(x, np.float32)
    src = np.asarray(src, np.int64)
    dst = np.asarray(dst, np.int64)
    cand_u = np.asarray(cand_u, np.int64)
    cand_v = np.asarray(cand_v, np.int64)
    cand_feat = np.asarray(cand_feat, np.float32)

    deg = np.bincount(dst, minlength=N).astype(np.float32)
    invdeg = 1.0 / np.maximum(deg, 1.0)
    em, edata = _prep_edges(src, dst, invdeg)
    cm, cdata = _prep_cands(cand_u, cand_v, cand_feat)

    xpad = np.zeros((NTOT, D), np.float32)
    xpad[:N] = x
    xperm = np.zeros((NTOT, D), np.float32)
    xperm[_newrow(np.arange(NTOT))] = xpad
    ident = np.eye(P, dtype=np.float32)
    onesr = np.ones((1, P), np.float32)

    com = {"xpad": xperm.astype(NPBF), "ident": ident, "onesr": onesr}
    for l, (ws, wn, b, ga, be, rme, rve) in enumerate(
        ((w_self0, w_neigh0, b0, gamma0, beta0, rm0, rv0),
         (w_self1, w_neigh1, b1, gamma1, beta1, rm1, rv1))):
        a = (ga / np.sqrt(rve + BN_EPS)).astype(np.float32)
        com[f"wself{l}"] = (ws * a[None, :]).astype(np.float32)
        com[f"wneigh{l}"] = (wn * a[None, :]).astype(np.float32)
        com[f"crow{l}"] = (a * (b - rme) + be).astype(np.float32)[None, :]
    com["amat"] = np.asarray(mw0[0:128], np.float32)
    com["bmat"] = np.asarray(mw0[128:256], np.float32)
    com["mb0r"] = np.asarray(mb0, np.float32)[None, :]
    com["w0r1"] = np.asarray(mw0[256], np.float32)[None, :].astype(NPBF)
    com["i64"] = np.eye(64, dtype=np.float32).astype(NPBF)
    com["mw1b"] = np.asarray(mw1, np.float32).astype(NPBF)
    com["mb1r1"] = np.asarray(mb1, np.float32)[None, :].astype(NPBF)
    com["mw2c"] = np.asarray(mw2, np.float32).astype(NPBF)
    com["mb2v"] = np.full((P, 1), np.float32(np.asarray(mb2).ravel()[0]))
    com["ones512"] = np.ones((1, 512), np.float32).astype(NPBF)

    nc = _build_nc(em, cm)
    in_maps = []
    for k in range(NCORE):
        m = dict(com)
        m["xT"] = xpad[k * NSH : (k + 1) * NSH].T.copy()
        m["gidx"] = edata[k]["gidx"]
        m["ohtab"] = edata[k]["ohtab"]
        m["cu"] = cdata[k]["cu"]
        m["cv"] = cdata[k]["cv"]
        m["featrow"] = cdata[k]["featrow"]
        m["maskr"] = cdata[k]["mask"]
        in_maps.append(m)
    import os
    trace = bool(os.environ.get("KERNEL_TRACE"))
    if trace:
        import types
        import ctypes
        if "antenv.axon_hooks" not in sys.modules:
            try:
                import antenv
                from trn_agent_boot.trn_boot import _ntff_profile_via_ctypes
                mod = types.ModuleType("antenv.axon_hooks")
                hook = [_ntff_profile_via_ctypes("/opt/axon/libaxon_pjrt.so")]
                mod.set_axon_ntff_profile_hook = lambda h: hook.__setitem__(0, h)
                mod.get_axon_ntff_profile_hook = lambda: hook[0]
                sys.modules["antenv.axon_hooks"] = mod
                antenv.axon_hooks = mod
            except Exception:
                trace = False
    res = run_bass_kernel_spmd(nc, in_maps, core_ids=list(range(NCORE)),
                               trace=trace,
                               tmpdir=os.environ.get("KERNEL_TRACE_DIR"))
    if trace and res.exec_time_ns is not None:
        print(f"HW exec time: {res.exec_time_ns} ns")
    y_all = np.zeros(C, np.float32)
    p_all = np.zeros(C, np.float32)
    ncc = cm["ncc"]
    p_lin = res.results[0]["p_out"].ravel()   # global order: k, p, c
    for k in range(NCORE):
        sm = cdata[k]["slotmap"]
        valid = sm >= 0
        j = np.nonzero(valid)[0]              # slot j = c*128 + p
        yk = res.results[k]["y_out"]          # [128, NCC] -> value at [j%128, j//128]
        y_all[sm[valid]] = yk[j % P, j // P]
        gs = k * cm["cslot"] + (j % P) * ncc + (j // P)
        p_all[sm[valid]] = p_lin[gs]
    return y_all[:, None], p_all[:, None]
